# revision 16
# baseline (speedup 1.0000x reference)
"""Multi-head self-attention (no mask) on 8 TRN2 NeuronCores — v3.

Problem: B=2, T=2048, C=1024, H=16 heads, D=64.
    q/k/v = x @ W{q,k,v}.T + b;  att = softmax(q k^T / sqrt(D));
    y = att v;  out = y @ Wp.T + bp.

Sharding: core (b, g) = batch x head-group of 4 heads.  Each core computes
q/k/v for its 4 heads, attention, and the partial output projection through
its 256 columns of Wp; the host sums 4 partials per batch and adds bp.

Structure:
  - Host pre-transposes and pre-casts to bf16 (x^T, W^T) — no input
    transposes on the PE, and half the input DMA bytes.
  - All matmuls bf16 (1 cyc/row on the PE), PSUM accumulation fp32.
  - exp(S/8 - ln16) on ACT (the 16.8M-element exp stream is the critical
    resource) written directly as bf16 P tiles.
  - y' = P^T V with stationary [V_h | ones] (65 cols) and moving P
    (512-wide streams): big streams amortize the ~200ns/instr PE overhead,
    and the ones column accumulates softmax denominators in PSUM row 64.
  - S(s+1) is emitted BEFORE y'(s) so the in-order PE always has
    exp-independent work queued while ACT computes exp(s); continuous PE
    execution also lets the tensor engine ramp out of its low p-state.
  - JIT emission interleaves k/q chunk projections and v projections into
    the attention s-loop so ACT starts at ~6us and stays fed.
"""

import sys
from contextlib import ExitStack

import numpy as np
import ml_dtypes

if "/opt/trn_rl_repo" not in sys.path:
    sys.path.insert(0, "/opt/trn_rl_repo")

import concourse.bass as bass  # noqa: F401
import concourse.mybir as mybir
import concourse.tile as tile
from concourse import bacc
from concourse.bass_utils import run_bass_kernel_spmd

F32 = mybir.dt.float32
BF16 = mybir.dt.bfloat16
Act = mybir.ActivationFunctionType

NP_BF16 = ml_dtypes.bfloat16

P = 128
B, C, HEADS, D = 2, 1024, 16, 64
GROUPS = 4            # head groups (one per core within a batch)
HLOC = HEADS // GROUPS
G = HLOC * D          # 256 channels per core
KT = C // P           # 8 contraction chunks
VW = D + 1            # v + ones column
LN16 = float(np.log(16.0))


def build(T=2048):
    TQ = 512
    NTQ = T // TQ         # 4 query chunks
    NS = T // P           # 16 key tiles
    NQB = TQ // P         # 4 query blocks per chunk

    nc = bacc.Bacc("TRN2", target_bir_lowering=False, debug=False)
    xT = nc.dram_tensor("xt", [C, T], BF16, kind="ExternalInput")
    wq = nc.dram_tensor("wq", [C, G], BF16, kind="ExternalInput")
    wk = nc.dram_tensor("wk", [C, G], BF16, kind="ExternalInput")
    wv = nc.dram_tensor("wv", [C, G], BF16, kind="ExternalInput")
    wp = nc.dram_tensor("wp", [G, C], BF16, kind="ExternalInput")
    bq = nc.dram_tensor("bq", [G], F32, kind="ExternalInput")
    bk = nc.dram_tensor("bk", [G], F32, kind="ExternalInput")
    bv = nc.dram_tensor("bv", [G], BF16, kind="ExternalInput")
    out = nc.dram_tensor("out", [T, C], F32, kind="ExternalOutput")

    with tile.TileContext(nc) as tc, ExitStack() as ctx:
        persist = ctx.enter_context(tc.tile_pool(name="persist", bufs=1))

        xs = persist.tile([P, KT, T], BF16, tag="xs")
        wq_sb = persist.tile([P, KT, G], BF16, tag="wq_sb")
        wk_sb = persist.tile([P, KT, G], BF16, tag="wk_sb")
        wv_sb = persist.tile([P, KT, G], BF16, tag="wv_sb")
        wp_sb = persist.tile([P, 2, C], BF16, tag="wp_sb")
        bq_pp = persist.tile([P, 2], F32, tag="bq_pp")
        bk_pp = persist.tile([P, 2], F32, tag="bk_pp")
        bv_row = persist.tile([1, G], BF16, tag="bv_row")
        ones_col = persist.tile([1, P], BF16, tag="ones_col")
        expb = persist.tile([P, 1], F32, tag="expb")
        qT = persist.tile([P, 2, T], BF16, tag="qT")
        kT = persist.tile([P, 2, T], BF16, tag="kT")
        v_sb = persist.tile([P, NS, HLOC, VW], BF16, tag="v_sb")
        yT = persist.tile([P, 2, T], BF16, tag="yT")

        nc.gpsimd.memset(ones_col[:], 1.0)
        nc.gpsimd.memset(expb[:], -LN16)
        nc.gpsimd.memset(v_sb[:], 1.0)  # ones column; v copies overwrite 0:D
        nc.sync.dma_start(bq_pp[:], bq[:].rearrange("(m p) -> p m", p=P))
        nc.sync.dma_start(bk_pp[:], bk[:].rearrange("(m p) -> p m", p=P))
        nc.sync.dma_start(bv_row[:], bv[None, :])
        nc.sync.dma_start(wq_sb[:], wq[:, :].rearrange("(k p) g -> p k g", p=P))
        nc.sync.dma_start(wk_sb[:], wk[:, :].rearrange("(k p) g -> p k g", p=P))
        nc.sync.dma_start(wv_sb[:], wv[:, :].rearrange("(k p) g -> p k g", p=P))
        nc.sync.dma_start(wp_sb[:], wp[:, :].rearrange("(m p) c -> p m c", p=P))
        for c in range(NTQ):
            ts = slice(c * TQ, (c + 1) * TQ)
            nc.sync.dma_start(
                xs[:, :, ts], xT[:, ts].rearrange("(k p) t -> p k t", p=P)
            )

        with (
            tc.tile_pool(name="psA", bufs=2, space="PSUM") as psA,
            tc.tile_pool(name="psY", bufs=2, space="PSUM") as psY,
            tc.tile_pool(name="ptp", bufs=3) as ptp,
            tc.tile_pool(name="npool", bufs=2) as npool,
            tc.tile_pool(name="obuf", bufs=2) as obuf,
        ):
            def proj_qk(m, cq, which, on_act):
                """q or k projection chunk: qT/kT[:, m, cq*TQ:...]."""
                w_sb, b_pp, dst = (
                    (wq_sb, bq_pp, qT) if which == 0 else (wk_sb, bk_pp, kT)
                )
                ts = slice(cq * TQ, (cq + 1) * TQ)
                pq = psA.tile([P, TQ], F32, tag="big")
                for kk in range(KT):
                    nc.tensor.matmul(
                        pq[:],
                        w_sb[:, kk, m * P : (m + 1) * P],
                        xs[:, kk, ts],
                        start=(kk == 0),
                        stop=(kk == KT - 1),
                    )
                if on_act:
                    nc.scalar.activation(
                        dst[:, m, ts], pq[:], Act.Identity,
                        bias=b_pp[:, m : m + 1], scale=1.0,
                    )
                else:
                    nc.vector.tensor_scalar_add(
                        dst[:, m, ts], pq[:], b_pp[:, m : m + 1]
                    )

            def proj_v(u):
                """v for key tiles (2u, 2u+1): v_sb[:, s, h, 0:D]."""
                pv = psA.tile([P, 2 * G], F32, tag="big")
                for i in range(2):
                    s = 2 * u + i
                    cs = slice(i * G, (i + 1) * G)
                    for kk in range(KT):
                        nc.tensor.matmul(
                            pv[:, cs],
                            xs[:, kk, s * P : (s + 1) * P],
                            wv_sb[:, kk, :],
                            start=(i == 0 and kk == 0),
                            stop=False,
                            skip_group_check=True,
                        )
                    nc.tensor.matmul(
                        pv[:, cs], ones_col[0:1, :], bv_row[0:1, :],
                        start=False, stop=(i == 1),
                        skip_group_check=True,
                    )
                    nc.vector.tensor_copy(
                        v_sb[:, s, :, 0:D],
                        pv[:, cs].rearrange("p (h d) -> p h d", d=D),
                    )

            def oproj(mt):
                ob = obuf.tile([P, C], F32, tag="ob")
                for n in range(2):
                    po = psA.tile([P, 512], F32, tag="big")
                    for j in range(2):
                        nc.tensor.matmul(
                            po[:],
                            yT[:, j, mt * P : (mt + 1) * P],
                            wp_sb[:, j, n * 512 : (n + 1) * 512],
                            start=(j == 0),
                            stop=(j == 1),
                        )
                    nc.vector.tensor_copy(ob[:, n * 512 : (n + 1) * 512], po[:])
                nc.sync.dma_start(out[mt * P : (mt + 1) * P, :], ob[:])

            # JIT emission schedule: extras[(pi, tq, s)] = thunks run at the
            # top of that attention s iteration (PE program order).
            extras = {
                (0, 0, 0): [lambda: proj_v(1)],
                (0, 0, 2): [lambda: proj_v(2), lambda: proj_qk(0, 1, 1, True)],
                (0, 0, 4): [lambda: proj_v(3)],
                (0, 0, 6): [lambda: proj_qk(0, 1, 0, True), lambda: proj_v(4)],
                (0, 0, 8): [lambda: proj_qk(0, 2, 1, True), lambda: proj_v(5)],
                (0, 0, 10): [lambda: proj_v(6)],
                (0, 0, 12): [lambda: proj_qk(0, 3, 1, True), lambda: proj_v(7)],
                (0, 0, 14): [lambda: proj_qk(0, 2, 0, False)],
                (0, 1, 0): [lambda: proj_qk(0, 3, 0, False)],
                (0, 1, 4): [lambda: proj_qk(1, 0, 1, False)],
                (0, 1, 8): [lambda: proj_qk(1, 0, 0, False)],
                (0, 1, 12): [lambda: proj_qk(1, 1, 1, False)],
                (0, 2, 2): [lambda: proj_qk(1, 1, 0, False)],
                (0, 2, 6): [lambda: proj_qk(1, 2, 1, False)],
                (0, 2, 10): [lambda: proj_qk(1, 2, 0, False)],
                (0, 2, 14): [lambda: proj_qk(1, 3, 1, False)],
                (0, 3, 0): [lambda: proj_qk(1, 3, 0, False)],
            }

            # prologue: k/q chunk 0 for head-pair 0 and the first v pair
            proj_qk(0, 0, 1, True)
            proj_qk(0, 0, 0, True)
            proj_v(0)

            def s_mm(pi, tq, s):
                ts = slice(tq * TQ, (tq + 1) * TQ)
                sp = psA.tile([P, 2 * TQ], F32, tag="big")
                for hh in range(2):
                    bp_ = 64 * hh
                    nc.tensor.matmul(
                        sp[:, hh * TQ : (hh + 1) * TQ],
                        kT[bp_ : bp_ + 64, pi, s * P : (s + 1) * P],
                        qT[bp_ : bp_ + 64, pi, ts],
                        start=True,
                        stop=True,
                    )
                # P = exp(S/8 - ln16) in bf16 (the shift cancels in y'/sigma)
                pt = ptp.tile([P, 2, TQ], BF16, tag="pt")
                nc.scalar.activation(
                    pt[:], sp[:], Act.Exp, bias=expb[:, 0:1], scale=0.125,
                )
                return pt

            for pi in range(2):
                for tq in range(NTQ):
                    ts = slice(tq * TQ, (tq + 1) * TQ)
                    py0 = psY.tile([VW, TQ], F32, tag="py0")
                    py1 = psY.tile([VW, TQ], F32, tag="py1")
                    pts = {}
                    for fn in extras.get((pi, tq, 0), ()):
                        fn()
                    pts[0] = s_mm(pi, tq, 0)
                    for s in range(NS):
                        # prefetch S(s+1): exp-independent PE work queued
                        # while ACT computes exp(s)
                        if s + 1 < NS:
                            for fn in extras.get((pi, tq, s + 1), ()):
                                fn()
                            pts[s + 1] = s_mm(pi, tq, s + 1)
                        pt = pts.pop(s)
                        for hh in range(2):
                            h = 2 * pi + hh
                            nc.tensor.matmul(
                                (py0, py1)[hh][:],
                                v_sb[:, s, h, :],
                                pt[:, hh, :],
                                start=(s == 0),
                                stop=(s == NS - 1),
                            )
                    # normalize: y_h / sigma_h (sigma in PSUM row 64)
                    for hh in range(2):
                        py = (py0, py1)[hh]
                        srow = npool.tile([VW, TQ], F32, tag=f"srow{hh}")
                        nc.vector.tensor_copy(srow[D : D + 1, :], py[D : D + 1, :])
                        srow0 = npool.tile([1, TQ], F32, tag=f"srow0{hh}")
                        nc.sync.dma_start(srow0[:], srow[D : D + 1, :])
                        recip0 = npool.tile([1, TQ], F32, tag=f"recip0{hh}")
                        nc.vector.reciprocal(recip0[0:1, :], srow0[0:1, :])
                        bcast = npool.tile([D, TQ], F32, tag=f"bcast{hh}")
                        nc.gpsimd.partition_broadcast(
                            bcast[:, :], recip0[0:1, :], channels=D
                        )
                        if hh == 0:
                            nc.vector.tensor_mul(
                                yT[0:D, pi, ts], py[0:D, :], bcast[:, :]
                            )
                        else:
                            y_tmp = npool.tile([D, TQ], BF16, tag="y_tmp")
                            nc.vector.tensor_mul(y_tmp[:], py[0:D, :], bcast[:, :])
                            nc.sync.dma_start(yT[D : 2 * D, pi, ts], y_tmp[:])

                    if pi == 1:
                        for w in range(T // P // NTQ):
                            oproj(tq * (T // P // NTQ) + w)

    nc.finalize()
    return nc


_NC_CACHE = {}


def _get_nc(T=2048):
    if T not in _NC_CACHE:
        _NC_CACHE[T] = build(T=T)
    return _NC_CACHE[T]


def _make_in_maps(x, Wq, bq, Wk, bk, Wv, bv, Wp):
    in_maps = []
    for b in range(B):
        xt = np.ascontiguousarray(x[b].T).astype(NP_BF16)
        for g in range(GROUPS):
            sl = slice(g * G, (g + 1) * G)
            in_maps.append(
                {
                    "xt": xt,
                    "wq": np.ascontiguousarray(Wq[sl, :].T).astype(NP_BF16),
                    "wk": np.ascontiguousarray(Wk[sl, :].T).astype(NP_BF16),
                    "wv": np.ascontiguousarray(Wv[sl, :].T).astype(NP_BF16),
                    "wp": np.ascontiguousarray(Wp[:, sl].T).astype(NP_BF16),
                    "bq": np.ascontiguousarray(bq[sl], dtype=np.float32),
                    "bk": np.ascontiguousarray(bk[sl], dtype=np.float32),
                    "bv": np.ascontiguousarray(bv[sl]).astype(NP_BF16),
                }
            )
    return in_maps


def run(inputs, trace=False):
    """Run on 8 cores; returns (out [B,T,C] fp32, BassKernelResults)."""
    x = np.asarray(inputs["x"], dtype=np.float32)
    T = x.shape[1]
    in_maps = _make_in_maps(
        x,
        np.asarray(inputs["Wq"]), np.asarray(inputs["bq"]),
        np.asarray(inputs["Wk"]), np.asarray(inputs["bk"]),
        np.asarray(inputs["Wv"]), np.asarray(inputs["bv"]),
        np.asarray(inputs["Wp"]),
    )
    nc = _get_nc(T)
    res = run_bass_kernel_spmd(
        nc, in_maps, core_ids=list(range(B * GROUPS)), trace=trace
    )
    bp = np.asarray(inputs["bp"], dtype=np.float32)
    parts = [res.results[i]["out"] for i in range(B * GROUPS)]
    out = np.stack(
        [sum(parts[b * GROUPS : (b + 1) * GROUPS]) for b in range(B)]
    ) + bp[None, None, :]
    return out.astype(np.float32), res


def kernel(**inputs):
    out, _ = run(inputs, trace=False)
    return out


# revision 18
# speedup vs baseline: 1.0827x; 1.0827x over previous
"""Multi-head self-attention (no mask) on 8 TRN2 NeuronCores — v3.

Problem: B=2, T=2048, C=1024, H=16 heads, D=64.
    q/k/v = x @ W{q,k,v}.T + b;  att = softmax(q k^T / sqrt(D));
    y = att v;  out = y @ Wp.T + bp.

Sharding: core (b, g) = batch x head-group of 4 heads.  Each core computes
q/k/v for its 4 heads, attention, and the partial output projection through
its 256 columns of Wp; the host sums 4 partials per batch and adds bp.

Structure:
  - Host pre-transposes and pre-casts to bf16 (x^T, W^T) — no input
    transposes on the PE, and half the input DMA bytes.
  - All matmuls bf16 (1 cyc/row on the PE), PSUM accumulation fp32.
  - exp(S/8 - ln16) on ACT (the 16.8M-element exp stream is the critical
    resource) written directly as bf16 P tiles.
  - y' = P^T V with stationary [V_h | ones] (65 cols) and moving P
    (512-wide streams): big streams amortize the ~200ns/instr PE overhead,
    and the ones column accumulates softmax denominators in PSUM row 64.
  - S(s+1) is emitted BEFORE y'(s) so the in-order PE always has
    exp-independent work queued while ACT computes exp(s); continuous PE
    execution also lets the tensor engine ramp out of its low p-state.
  - JIT emission interleaves k/q chunk projections and v projections into
    the attention s-loop so ACT starts at ~6us and stays fed.
"""

import sys
from contextlib import ExitStack

import numpy as np
import ml_dtypes

if "/opt/trn_rl_repo" not in sys.path:
    sys.path.insert(0, "/opt/trn_rl_repo")

import concourse.bass as bass  # noqa: F401
import concourse.mybir as mybir
import concourse.tile as tile
from concourse import bacc
from concourse.bass_utils import run_bass_kernel_spmd

F32 = mybir.dt.float32
BF16 = mybir.dt.bfloat16
Act = mybir.ActivationFunctionType

NP_BF16 = ml_dtypes.bfloat16

P = 128
B, C, HEADS, D = 2, 1024, 16, 64
GROUPS = 4            # head groups (one per core within a batch)
HLOC = HEADS // GROUPS
G = HLOC * D          # 256 channels per core
KT = C // P           # 8 contraction chunks
VW = D + 1            # v + ones column
LN16 = float(np.log(16.0))


def build(T=2048):
    TQ = 512
    NTQ = T // TQ         # 4 query chunks
    NS = T // P           # 16 key tiles
    NQB = TQ // P         # 4 query blocks per chunk

    nc = bacc.Bacc("TRN2", target_bir_lowering=False, debug=False)
    xT = nc.dram_tensor("xt", [C, T], BF16, kind="ExternalInput")
    wq = nc.dram_tensor("wq", [C, G], BF16, kind="ExternalInput")
    wk = nc.dram_tensor("wk", [C, G], BF16, kind="ExternalInput")
    wv = nc.dram_tensor("wv", [C, G], BF16, kind="ExternalInput")
    wp = nc.dram_tensor("wp", [G, C], BF16, kind="ExternalInput")
    bq = nc.dram_tensor("bq", [G], F32, kind="ExternalInput")
    bk = nc.dram_tensor("bk", [G], F32, kind="ExternalInput")
    bv = nc.dram_tensor("bv", [G], BF16, kind="ExternalInput")
    out = nc.dram_tensor("out", [T, C], F32, kind="ExternalOutput")

    with tile.TileContext(nc) as tc, ExitStack() as ctx:
        persist = ctx.enter_context(tc.tile_pool(name="persist", bufs=1))

        xs = persist.tile([P, KT, T], BF16, tag="xs")
        wq_sb = persist.tile([P, KT, G], BF16, tag="wq_sb")
        wk_sb = persist.tile([P, KT, G], BF16, tag="wk_sb")
        wv_sb = persist.tile([P, KT, G], BF16, tag="wv_sb")
        wp_sb = persist.tile([P, 2, C], BF16, tag="wp_sb")
        bq_pp = persist.tile([P, 2], F32, tag="bq_pp")
        bk_pp = persist.tile([P, 2], F32, tag="bk_pp")
        bv_row = persist.tile([1, G], BF16, tag="bv_row")
        ones_col = persist.tile([1, P], BF16, tag="ones_col")
        expb = persist.tile([P, 1], F32, tag="expb")
        qT = persist.tile([P, 2, T], BF16, tag="qT")
        kT = persist.tile([P, 2, T], BF16, tag="kT")
        v_sb = persist.tile([P, NS, HLOC, VW], BF16, tag="v_sb")
        yT = persist.tile([P, 2, T], BF16, tag="yT")

        nc.gpsimd.memset(ones_col[:], 1.0)
        nc.gpsimd.memset(expb[:], -LN16)
        nc.gpsimd.memset(v_sb[:], 1.0)  # ones column; v copies overwrite 0:D
        def dma_x(c):
            ts = slice(c * TQ, (c + 1) * TQ)
            nc.sync.dma_start(
                xs[:, :, ts], xT[:, ts].rearrange("(k p) t -> p k t", p=P)
            )

        dma_x(0)
        nc.sync.dma_start(wk_sb[:], wk[:, :].rearrange("(k p) g -> p k g", p=P))
        nc.sync.dma_start(wq_sb[:], wq[:, :].rearrange("(k p) g -> p k g", p=P))
        nc.sync.dma_start(wv_sb[:], wv[:, :].rearrange("(k p) g -> p k g", p=P))
        nc.sync.dma_start(bq_pp[:], bq[:].rearrange("(m p) -> p m", p=P))
        nc.sync.dma_start(bk_pp[:], bk[:].rearrange("(m p) -> p m", p=P))
        nc.sync.dma_start(bv_row[:], bv[None, :])
        for c in range(1, NTQ):
            dma_x(c)
        nc.sync.dma_start(wp_sb[:], wp[:, :].rearrange("(m p) c -> p m c", p=P))

        with (
            tc.tile_pool(name="psA", bufs=2, space="PSUM") as psA,
            tc.tile_pool(name="psY", bufs=2, space="PSUM") as psY,
            tc.tile_pool(name="ptp", bufs=3) as ptp,
            tc.tile_pool(name="npool", bufs=2) as npool,
            tc.tile_pool(name="obuf", bufs=2) as obuf,
        ):
            def proj_qk(m, cq, which, on_act):
                """q or k projection chunk: qT/kT[:, m, cq*TQ:...]."""
                w_sb, b_pp, dst = (
                    (wq_sb, bq_pp, qT) if which == 0 else (wk_sb, bk_pp, kT)
                )
                ts = slice(cq * TQ, (cq + 1) * TQ)
                pq = psA.tile([P, TQ], F32, tag="big")
                for kk in range(KT):
                    nc.tensor.matmul(
                        pq[:],
                        w_sb[:, kk, m * P : (m + 1) * P],
                        xs[:, kk, ts],
                        start=(kk == 0),
                        stop=(kk == KT - 1),
                    )
                if on_act:
                    nc.scalar.activation(
                        dst[:, m, ts], pq[:], Act.Identity,
                        bias=b_pp[:, m : m + 1], scale=1.0,
                    )
                else:
                    nc.vector.tensor_scalar_add(
                        dst[:, m, ts], pq[:], b_pp[:, m : m + 1]
                    )

            def proj_v(u):
                """v for key tiles (2u, 2u+1): v_sb[:, s, h, 0:D]."""
                pv = psA.tile([P, 2 * G], F32, tag="big")
                for i in range(2):
                    s = 2 * u + i
                    cs = slice(i * G, (i + 1) * G)
                    for kk in range(KT):
                        nc.tensor.matmul(
                            pv[:, cs],
                            xs[:, kk, s * P : (s + 1) * P],
                            wv_sb[:, kk, :],
                            start=(i == 0 and kk == 0),
                            stop=False,
                            skip_group_check=True,
                        )
                    nc.tensor.matmul(
                        pv[:, cs], ones_col[0:1, :], bv_row[0:1, :],
                        start=False, stop=(i == 1),
                        skip_group_check=True,
                    )
                    nc.vector.tensor_copy(
                        v_sb[:, s, :, 0:D],
                        pv[:, cs].rearrange("p (h d) -> p h d", d=D),
                    )

            def oproj(mt):
                ob = obuf.tile([P, C], F32, tag="ob")
                for n in range(2):
                    po = psA.tile([P, 512], F32, tag="big")
                    for j in range(2):
                        nc.tensor.matmul(
                            po[:],
                            yT[:, j, mt * P : (mt + 1) * P],
                            wp_sb[:, j, n * 512 : (n + 1) * 512],
                            start=(j == 0),
                            stop=(j == 1),
                        )
                    nc.vector.tensor_copy(ob[:, n * 512 : (n + 1) * 512], po[:])
                nc.sync.dma_start(out[mt * P : (mt + 1) * P, :], ob[:])

            # JIT emission schedule: extras[(pi, tq, s)] = thunks run at the
            # top of that attention s iteration (PE program order).
            extras = {
                (0, 0, 0): [lambda: proj_v(1)],
                (0, 0, 2): [lambda: proj_v(2), lambda: proj_qk(0, 1, 1, True)],
                (0, 0, 4): [lambda: proj_v(3)],
                (0, 0, 6): [lambda: proj_qk(0, 1, 0, True), lambda: proj_v(4)],
                (0, 0, 8): [lambda: proj_qk(0, 2, 1, True), lambda: proj_v(5)],
                (0, 0, 10): [lambda: proj_v(6)],
                (0, 0, 12): [lambda: proj_qk(0, 3, 1, True), lambda: proj_v(7)],
                (0, 0, 14): [lambda: proj_qk(0, 2, 0, False)],
                (0, 1, 0): [lambda: proj_qk(0, 3, 0, False)],
                (0, 1, 4): [lambda: proj_qk(1, 0, 1, False)],
                (0, 1, 8): [lambda: proj_qk(1, 0, 0, False)],
                (0, 1, 12): [lambda: proj_qk(1, 1, 1, False)],
                (0, 2, 2): [lambda: proj_qk(1, 1, 0, False)],
                (0, 2, 6): [lambda: proj_qk(1, 2, 1, False)],
                (0, 2, 10): [lambda: proj_qk(1, 2, 0, False)],
                (0, 2, 14): [lambda: proj_qk(1, 3, 1, False)],
                (0, 3, 0): [lambda: proj_qk(1, 3, 0, False)],
            }

            # prologue: k/q chunk 0 for head-pair 0 and the first v pair
            proj_qk(0, 0, 1, True)
            proj_qk(0, 0, 0, True)
            proj_v(0)

            def s_mm(pi, tq, s):
                ts = slice(tq * TQ, (tq + 1) * TQ)
                sp = psA.tile([P, 2 * TQ], F32, tag="big")
                for hh in range(2):
                    bp_ = 64 * hh
                    nc.tensor.matmul(
                        sp[:, hh * TQ : (hh + 1) * TQ],
                        kT[bp_ : bp_ + 64, pi, s * P : (s + 1) * P],
                        qT[bp_ : bp_ + 64, pi, ts],
                        start=True,
                        stop=True,
                    )
                # P = exp(S/8 - ln16) in bf16 (the shift cancels in y'/sigma)
                pt = ptp.tile([P, 2, TQ], BF16, tag="pt")
                nc.scalar.activation(
                    pt[:], sp[:], Act.Exp, bias=expb[:, 0:1], scale=0.125,
                )
                return pt

            for pi in range(2):
                for tq in range(NTQ):
                    ts = slice(tq * TQ, (tq + 1) * TQ)
                    py0 = psY.tile([VW, TQ], F32, tag="py0")
                    py1 = psY.tile([VW, TQ], F32, tag="py1")
                    pts = {}
                    for fn in extras.get((pi, tq, 0), ()):
                        fn()
                    pts[0] = s_mm(pi, tq, 0)
                    for s in range(NS):
                        # deferred out-projection of the previous tq (pi=1):
                        # spread over this tq's s-loop so the PE never waits
                        # on the normalization chain
                        if pi == 1 and tq > 0 and s in (4, 7, 10, 13):
                            oproj((tq - 1) * 4 + (s - 4) // 3)
                        # prefetch S(s+1): exp-independent PE work queued
                        # while ACT computes exp(s)
                        if s + 1 < NS:
                            for fn in extras.get((pi, tq, s + 1), ()):
                                fn()
                            pts[s + 1] = s_mm(pi, tq, s + 1)
                        pt = pts.pop(s)
                        for hh in range(2):
                            h = 2 * pi + hh
                            nc.tensor.matmul(
                                (py0, py1)[hh][:],
                                v_sb[:, s, h, :],
                                pt[:, hh, :],
                                start=(s == 0),
                                stop=(s == NS - 1),
                            )
                    # normalize: y_h / sigma_h (sigma in PSUM row 64)
                    for hh in range(2):
                        py = (py0, py1)[hh]
                        srow = npool.tile([VW, TQ], F32, tag=f"srow{hh}")
                        nc.vector.tensor_copy(srow[D : D + 1, :], py[D : D + 1, :])
                        srow0 = npool.tile([1, TQ], F32, tag=f"srow0{hh}")
                        nc.sync.dma_start(srow0[:], srow[D : D + 1, :])
                        recip0 = npool.tile([1, TQ], F32, tag=f"recip0{hh}")
                        nc.vector.reciprocal(recip0[0:1, :], srow0[0:1, :])
                        bcast = npool.tile([D, TQ], F32, tag=f"bcast{hh}")
                        nc.gpsimd.partition_broadcast(
                            bcast[:, :], recip0[0:1, :], channels=D
                        )
                        if hh == 0:
                            nc.vector.tensor_mul(
                                yT[0:D, pi, ts], py[0:D, :], bcast[:, :]
                            )
                        else:
                            y_tmp = npool.tile([D, TQ], BF16, tag="y_tmp")
                            nc.vector.tensor_mul(y_tmp[:], py[0:D, :], bcast[:, :])
                            nc.sync.dma_start(yT[D : 2 * D, pi, ts], y_tmp[:])

            for w in range(4):
                oproj((NTQ - 1) * 4 + w)

    nc.finalize()
    return nc


_NC_CACHE = {}


def _get_nc(T=2048):
    if T not in _NC_CACHE:
        _NC_CACHE[T] = build(T=T)
    return _NC_CACHE[T]


def _make_in_maps(x, Wq, bq, Wk, bk, Wv, bv, Wp):
    in_maps = []
    for b in range(B):
        xt = np.ascontiguousarray(x[b].T).astype(NP_BF16)
        for g in range(GROUPS):
            sl = slice(g * G, (g + 1) * G)
            in_maps.append(
                {
                    "xt": xt,
                    "wq": np.ascontiguousarray(Wq[sl, :].T).astype(NP_BF16),
                    "wk": np.ascontiguousarray(Wk[sl, :].T).astype(NP_BF16),
                    "wv": np.ascontiguousarray(Wv[sl, :].T).astype(NP_BF16),
                    "wp": np.ascontiguousarray(Wp[:, sl].T).astype(NP_BF16),
                    "bq": np.ascontiguousarray(bq[sl], dtype=np.float32),
                    "bk": np.ascontiguousarray(bk[sl], dtype=np.float32),
                    "bv": np.ascontiguousarray(bv[sl]).astype(NP_BF16),
                }
            )
    return in_maps


def run(inputs, trace=False):
    """Run on 8 cores; returns (out [B,T,C] fp32, BassKernelResults)."""
    x = np.asarray(inputs["x"], dtype=np.float32)
    T = x.shape[1]
    in_maps = _make_in_maps(
        x,
        np.asarray(inputs["Wq"]), np.asarray(inputs["bq"]),
        np.asarray(inputs["Wk"]), np.asarray(inputs["bk"]),
        np.asarray(inputs["Wv"]), np.asarray(inputs["bv"]),
        np.asarray(inputs["Wp"]),
    )
    nc = _get_nc(T)
    res = run_bass_kernel_spmd(
        nc, in_maps, core_ids=list(range(B * GROUPS)), trace=trace
    )
    bp = np.asarray(inputs["bp"], dtype=np.float32)
    parts = [res.results[i]["out"] for i in range(B * GROUPS)]
    out = np.stack(
        [sum(parts[b * GROUPS : (b + 1) * GROUPS]) for b in range(B)]
    ) + bp[None, None, :]
    return out.astype(np.float32), res


def kernel(**inputs):
    out, _ = run(inputs, trace=False)
    return out
